# revision 31
# baseline (speedup 1.0000x reference)
"""Trainium2 Bass kernel for GNN multi-head attention (gnn_message_passing).

Reference computation (per problem):
    Q = h @ Wq + bq; K = h @ Wk + bk; V = h @ Wv + bv       [N, H*D]
    pe = e @ We + be                                         [E, H*D]
    score = (K[src] * Q[dst]) / sqrt(D) * pe                 [E, H, D]  -> e_out
    att = exp(clip(sum_d score, -5, 5))                      [E, H, 1]
    wV = segment_sum(V[src] * att, dst, N); z = segment_sum(att, dst, N)
    h_out = wV / (z + 1e-6)                                  [N, H, D]

Strategy: edges are sorted by dst on the host and the 40000 destination nodes
are sharded across the 8 cores (5000 each), so each core's incoming edges are
a contiguous range of the sorted edge list and the segment-sum is core-local.
Per 128-node block the one-hot M[e, n] = (dst[e] == block_base + n) drives both
directions on the tensor engine: M^T expands the block's Q tile to per-edge
rows (so there is no Q gather and no Q table at all -- dst is block-local),
and M accumulates [V*att | att] into PSUM for the segment sum. K/V live in one
interleaved DRAM table built in phase 1, so a single indirect DMA per edge
tile gathers both (src is global). e arrives pre-transposed and pre-divided by
sqrt(D)=4 (an exact power-of-two fold) so pe is a plain matmul with the edge
tile as the stationary operand and e_out keeps full fp32 fidelity.
"""

import math

import numpy as np

# problem constants (hardcoded per contract - kernel.py must be self-contained)
N = 40000
E = 640000
IN = 128
H = 8
D = 16
HD = H * D  # 128
P = 128
NCORES = 8
NODES_PER_CORE = N // NCORES  # 5000
BLOCKS_PER_CORE = math.ceil(NODES_PER_CORE / P)  # 40 (39 full + 1 of 8 nodes)
MACRO = 16  # edge tiles per macro (gather/load batch)
NODE_PAD = 40064  # 313 * 128, node-table row padding
PH1_TILES = NODE_PAD // P  # 313


def _build_program(t_blocks, use_bias):
    """Build the SPMD Bass program. t_blocks: per-block tile counts,
    identical across cores (len BLOCKS_PER_CORE)."""
    import concourse.bacc as bacc
    import concourse.bass as bass
    import concourse.mybir as mybir
    import concourse.tile as tile

    f32 = mybir.dt.float32
    i32 = mybir.dt.int32
    Alu = mybir.AluOpType

    t_total = sum(t_blocks)
    assert t_total % MACRO == 0 and MACRO % 4 == 0
    n_macros = t_total // MACRO
    S = t_total * P  # edge slots per core

    nc = bacc.Bacc()

    # ---- parameters ----
    hT = nc.declare_dram_parameter("hT", [P, NODE_PAD], f32, isOutput=False)
    hTblk = nc.declare_dram_parameter(
        "hTblk", [P, BLOCKS_PER_CORE * P], f32, isOutput=False
    )
    eT = nc.declare_dram_parameter("eT", [P, S], f32, isOutput=False)
    idx = nc.declare_dram_parameter(
        "idx", [n_macros, P, 2 * MACRO], i32, isOutput=False
    )
    wkv = nc.declare_dram_parameter("Wkv", [IN, 2 * HD], f32, isOutput=False)
    wq = nc.declare_dram_parameter("Wq", [IN, HD], f32, isOutput=False)
    we = nc.declare_dram_parameter("We", [IN, HD], f32, isOutput=False)
    iota = nc.declare_dram_parameter("iota", [P, P], f32, isOutput=False)
    ident = nc.declare_dram_parameter("ident", [P, P], f32, isOutput=False)
    if use_bias:
        # [bk | bv | bq | be/4]
        bias = nc.declare_dram_parameter("bias", [1, 4 * HD], f32, isOutput=False)
    eout = nc.declare_dram_parameter("eout", [S, HD], f32, isOutput=True)
    hout = nc.declare_dram_parameter("hout", [NODES_PER_CORE, HD], f32, isOutput=True)

    # ---- internal DRAM K|V node table ----
    kvtab = nc.dram_tensor("kvtab", [NODE_PAD, 2 * HD], f32)

    with tile.TileContext(nc) as tc:
        with tc.tile_pool(name="const", bufs=1) as cpool:
            wkv_sb = cpool.tile([IN, 2 * HD], f32)
            wq_sb = cpool.tile([IN, HD], f32)
            we_sb = cpool.tile([IN, HD], f32)
            iota_sb = cpool.tile([P, P], f32)
            ident_sb = cpool.tile([P, P], f32)
            nc.sync.dma_start(out=wkv_sb[:], in_=wkv[:])
            nc.sync.dma_start(out=wq_sb[:], in_=wq[:])
            nc.sync.dma_start(out=we_sb[:], in_=we[:])
            nc.sync.dma_start(out=iota_sb[:], in_=iota[:])
            nc.sync.dma_start(out=ident_sb[:], in_=ident[:])
            if use_bias:
                bias_sb = cpool.tile([1, 4 * HD], f32)
                nc.sync.dma_start(out=bias_sb[:], in_=bias[:])
                ones_sb = cpool.tile([1, P], f32)
                nc.vector.memset(ones_sb[:], 1.0)

            # ---------------- phase 1: K|V node table ----------------
            with (
                tc.tile_pool(name="p1sb", bufs=3) as p1sb,
                tc.tile_pool(name="p1ps", bufs=4, space="PSUM") as p1ps,
                tc.tile_pool(name="p1out", bufs=3) as p1out,
            ):
                HMAC = 16  # node tiles per hT load / store group
                for i in range(PH1_TILES):
                    j = i % HMAC
                    if j == 0:
                        grp = min(HMAC, PH1_TILES - i)
                        w = grp * P
                        hl = p1sb.tile([P, HMAC * P], f32, tag="hl")
                        leng = nc.sync if (i // HMAC) % 2 == 1 else nc.scalar
                        leng.dma_start(out=hl[:, :w], in_=hT[:, i * P : i * P + w])
                        ob = p1out.tile([P, HMAC, 2 * HD], f32, tag="ob")
                    ps = p1ps.tile([P, 2 * HD], f32, space="PSUM")
                    nc.tensor.matmul(
                        out=ps[:],
                        lhsT=hl[:, j * P : (j + 1) * P],
                        rhs=wkv_sb[:],
                        start=True,
                        stop=use_bias is False,
                    )
                    if use_bias:
                        nc.tensor.matmul(
                            out=ps[:],
                            lhsT=ones_sb[:],
                            rhs=bias_sb[:, 0 : 2 * HD],
                            start=False,
                            stop=True,
                        )
                    nc.vector.tensor_copy(out=ob[:, j, :], in_=ps[:])
                    if j == grp - 1:
                        r0 = (i - j) * P
                        eng = nc.sync if (i // HMAC) % 2 == 0 else nc.scalar
                        eng.dma_start(
                            out=kvtab[r0 : r0 + grp * P, :].rearrange(
                                "(c p) f -> p c f", p=P
                            ),
                            in_=ob[:, :grp, :],
                        )

            # ---------------- phase 2: edges ----------------
            with (
                tc.tile_pool(name="idxp", bufs=3) as idxp,
                tc.tile_pool(name="etp", bufs=4) as etp,
                tc.tile_pool(name="kvp", bufs=4) as kvp,
                tc.tile_pool(name="eop", bufs=3) as eop,
                tc.tile_pool(name="qbp", bufs=2) as qbp,
                tc.tile_pool(name="pep", bufs=2, space="PSUM") as pep,
                tc.tile_pool(name="qep", bufs=3, space="PSUM") as qep,
                tc.tile_pool(name="mtqb", bufs=2, space="PSUM") as mtqb,
                tc.tile_pool(name="wvzp", bufs=1, space="PSUM") as wvzp,
                tc.tile_pool(name="work", bufs=5) as work,
            ):
                state = {"stripe": None, "hb": None, "qblk": None, "qb_b": None,
                         "wv": None}

                def _ensure_qblk(b):
                    if state["qb_b"] == b:
                        return
                    stripe = b // 4
                    if state["stripe"] != stripe:
                        hb = qbp.tile([P, 4 * P], f32, tag="hb")
                        w = min(4 * P, BLOCKS_PER_CORE * P - stripe * 4 * P)
                        nc.sync.dma_start(
                            out=hb[:, :w],
                            in_=hTblk[:, stripe * 4 * P : stripe * 4 * P + w],
                        )
                        state["stripe"] = stripe
                        state["hb"] = hb
                    qps = mtqb.tile([P, HD], f32, space="PSUM", tag="mtqb")
                    nc.tensor.matmul(
                        out=qps[:],
                        lhsT=state["hb"][:, (b % 4) * P : (b % 4 + 1) * P],
                        rhs=wq_sb[:],
                        start=True,
                        stop=use_bias is False,
                    )
                    if use_bias:
                        nc.tensor.matmul(
                            out=qps[:],
                            lhsT=ones_sb[:],
                            rhs=bias_sb[:, 2 * HD : 3 * HD],
                            start=False,
                            stop=True,
                        )
                    qsb = qbp.tile([P, HD], f32, tag="qblk")
                    nc.scalar.copy(out=qsb[:], in_=qps[:])
                    state["qblk"] = qsb
                    state["qb_b"] = b

                def _emit_block_end(b):
                    # h_out = wV / (z + 1e-6) for node block b
                    wv_ps = state["wv"]
                    nrows = min(P, NODES_PER_CORE - b * P)
                    zt = work.tile([P, H], f32, tag="zt")
                    nc.vector.tensor_scalar(
                        out=zt[:],
                        in0=wv_ps[:, HD : HD + H],
                        scalar1=1e-6,
                        scalar2=None,
                        op0=Alu.add,
                    )
                    zr = work.tile([P, H], f32, tag="zr")
                    nc.vector.reciprocal(out=zr[:], in_=zt[:])
                    ho = work.tile([P, HD], f32, tag="ho")
                    nc.vector.tensor_tensor(
                        out=ho[:].rearrange("p (h d) -> p h d", d=D),
                        in0=wv_ps[:, 0:HD].rearrange("p (h d) -> p h d", d=D),
                        in1=zr[:, :, None].to_broadcast([P, H, D]),
                        op=Alu.mult,
                    )
                    nc.sync.dma_start(
                        out=hout[b * P : b * P + nrows, :], in_=ho[:nrows, :]
                    )

                # flat tile list: (block, t_within_block, tb)
                tinfo = []
                for b in range(BLOCKS_PER_CORE):
                    for t in range(t_blocks[b]):
                        tinfo.append((b, t, t_blocks[b]))
                assert len(tinfo) % 4 == 0

                idx_sb = eT_sb = kv_sb = eo_sb = None
                for g4 in range(len(tinfo) // 4):
                    g0 = g4 * 4
                    m, j0 = divmod(g0, MACRO)
                    if j0 == 0:
                        idx_sb = idxp.tile([P, 2 * MACRO], i32, tag="idx")
                        nc.sync.dma_start(out=idx_sb[:], in_=idx[m])
                        eT_sb = etp.tile([P, MACRO * P], f32, tag="et")
                        nc.sync.dma_start(
                            out=eT_sb[:],
                            in_=eT[:, m * MACRO * P : (m + 1) * MACRO * P],
                        )
                        kv_sb = kvp.tile([P, MACRO, 2 * HD], f32, tag="kv")
                        for jj in range(MACRO):
                            nc.gpsimd.indirect_dma_start(
                                out=kv_sb[:, jj, :],
                                out_offset=None,
                                in_=kvtab[:],
                                in_offset=bass.IndirectOffsetOnAxis(
                                    ap=idx_sb[:, jj : jj + 1], axis=0
                                ),
                            )
                        eo_sb = eop.tile([P, MACRO * P], f32, tag="eo")

                    pe_ps = pep.tile([P, 4 * HD], f32, space="PSUM", tag="pe")
                    qe_ps = qep.tile([P, 4 * HD], f32, space="PSUM", tag="qe")
                    m4 = work.tile([P, 4 * P], f32, tag="m4")
                    t4 = work.tile([P, 4 * P], f32, tag="t4")
                    va4 = work.tile([P, 4, HD + H], f32, tag="va4")
                    red4 = work.tile([P, 4 * H], f32, tag="red4")

                    # one-hot M (edge-major) and M^T (node-major) for 4 tiles
                    nc.vector.tensor_tensor(
                        out=m4[:].rearrange("p (c f) -> p c f", f=P),
                        in0=iota_sb[:, None, :].to_broadcast([P, 4, P]),
                        in1=idx_sb[:, MACRO + j0 : MACRO + j0 + 4]
                        .bitcast(f32)[:, :, None]
                        .to_broadcast([P, 4, P]),
                        op=Alu.is_equal,
                    )

                    for q4 in range(4):
                        j = j0 + q4
                        b, t, tb = tinfo[g0 + q4]
                        _ensure_qblk(b)
                        # pe' = (e/4) @ We
                        nc.tensor.matmul(
                            out=pe_ps[:, q4 * HD : (q4 + 1) * HD],
                            lhsT=eT_sb[:, j * P : (j + 1) * P],
                            rhs=we_sb[:],
                            start=True,
                            stop=use_bias is False,
                        )
                        if use_bias:
                            nc.tensor.matmul(
                                out=pe_ps[:, q4 * HD : (q4 + 1) * HD],
                                lhsT=ones_sb[:],
                                rhs=bias_sb[:, 3 * HD : 4 * HD],
                                start=False,
                                stop=True,
                            )
                        # M^T for this tile via PE transpose
                        mt_ps = mtqb.tile(
                            [P, P], f32, space="PSUM", tag="mtqb", name="mt_ps"
                        )
                        nc.tensor.transpose(
                            out=mt_ps[:],
                            in_=m4[:, q4 * P : (q4 + 1) * P],
                            identity=ident_sb[:],
                        )
                        mt_sb = work.tile([P, P], f32, tag="mt_sb")
                        nc.scalar.copy(out=mt_sb[:], in_=mt_ps[:])
                        # Q[dst] = M^T-expand of the block Q tile (exact row copy)
                        nc.tensor.matmul(
                            out=qe_ps[:, q4 * HD : (q4 + 1) * HD],
                            lhsT=mt_sb[:],
                            rhs=state["qblk"][:],
                            start=True,
                            stop=True,
                        )

                    # K*Q for 4 tiles
                    nc.vector.tensor_tensor(
                        out=t4[:].rearrange("p (c f) -> p c f", f=P),
                        in0=kv_sb[:, j0 : j0 + 4, 0:HD],
                        in1=qe_ps[:].rearrange("p (c f) -> p c f", f=P),
                        op=Alu.mult,
                    )
                    # score = (K*Q) * pe' for 4 tiles (this is the e_out data)
                    eo4 = eo_sb[:, j0 * P : (j0 + 4) * P]
                    nc.vector.tensor_tensor(
                        out=eo4, in0=t4[:], in1=pe_ps[:], op=Alu.mult
                    )
                    # att = exp(clip(sum_d score, -5, 5)) for 4 tiles
                    nc.vector.tensor_reduce(
                        out=red4[:],
                        in_=eo4.rearrange("p (c h d) -> p c h d", h=H, d=D),
                        axis=mybir.AxisListType.X,
                        op=Alu.add,
                    )
                    nc.vector.tensor_scalar(
                        out=red4[:],
                        in0=red4[:],
                        scalar1=5.0,
                        scalar2=-5.0,
                        op0=Alu.min,
                        op1=Alu.max,
                    )
                    nc.scalar.activation(
                        out=va4[:, :, HD : HD + H],
                        in_=red4[:].rearrange("p (c h) -> p c h", h=H),
                        func=mybir.ActivationFunctionType.Exp,
                    )
                    # V * att for 4 tiles (broadcast att over D)
                    nc.vector.tensor_tensor(
                        out=va4[:, :, 0:HD].rearrange("p c (h d) -> p c h d", d=D),
                        in0=kv_sb[:, j0 : j0 + 4, HD : 2 * HD].rearrange(
                            "p c (h d) -> p c h d", d=D
                        ),
                        in1=va4[:, :, HD : HD + H][:, :, :, None].to_broadcast(
                            [P, 4, H, D]
                        ),
                        op=Alu.mult,
                    )
                    if j0 == MACRO - 4:
                        # flush e_out macro: SBUF [P, MACRO, P] -> DRAM rows
                        nc.sync.dma_start(
                            out=eout[
                                m * MACRO * P : (m + 1) * MACRO * P, :
                            ].rearrange("(c p) f -> p c f", p=P),
                            in_=eo_sb[:].rearrange("p (c f) -> p c f", f=P),
                        )
                    # ---- per-tile segment-sum accumulate: [wV | z] ----
                    for q4 in range(4):
                        b, t, tb = tinfo[g0 + q4]
                        if t == 0:
                            state["wv"] = wvzp.tile(
                                [P, HD + H], f32, space="PSUM", tag="wv",
                                name="wv_ps",
                            )
                        last = t == tb - 1
                        nc.tensor.matmul(
                            out=state["wv"][:],
                            lhsT=m4[:, q4 * P : (q4 + 1) * P],
                            rhs=va4[:, q4, :],
                            start=(t == 0),
                            stop=last,
                        )
                        if last:
                            _emit_block_end(b)

    nc.compile()
    return nc


def _host_prep(h, e, src, dst):
    """Sort/shard on host. Returns (t_blocks, in_maps, eout_maps)."""
    src = np.ascontiguousarray(src.astype(np.int32))
    dst = np.ascontiguousarray(dst.astype(np.int32))
    h = np.ascontiguousarray(h.astype(np.float32))
    e = np.ascontiguousarray(e.astype(np.float32))

    perm = np.argsort(dst, kind="stable")
    dst_s = dst[perm]

    starts = np.empty((NCORES, BLOCKS_PER_CORE), dtype=np.int64)
    ends = np.empty((NCORES, BLOCKS_PER_CORE), dtype=np.int64)
    for c in range(NCORES):
        lo = c * NODES_PER_CORE
        hi = (c + 1) * NODES_PER_CORE
        bl = np.minimum(lo + np.arange(BLOCKS_PER_CORE) * P, hi)
        bh = np.minimum(bl + P, hi)
        starts[c] = np.searchsorted(dst_s, bl)
        ends[c] = np.searchsorted(dst_s, bh)
    cnt = ends - starts  # [NCORES, BLOCKS_PER_CORE]
    t_blocks = np.maximum(1, ((cnt + P - 1) // P).max(axis=0))
    t_total = int(t_blocks.sum())
    pad = (-t_total) % MACRO
    t_blocks[-1] += pad
    t_total += pad
    t_blocks = [int(x) for x in t_blocks]
    S = t_total * P

    iota_np = np.tile(np.arange(P, dtype=np.float32), (P, 1))
    ident_np = np.eye(P, dtype=np.float32)
    hT_np = np.zeros((P, NODE_PAD), dtype=np.float32)
    hT_np[:, :N] = h.T

    in_maps = []
    eout_maps = []
    block_off = np.concatenate([[0], np.cumsum(np.array(t_blocks) * P)])
    n_macros = S // (MACRO * P)
    for c in range(NCORES):
        slot_src = np.zeros(S, dtype=np.int32)
        slot_rel = np.full(S, -1.0, dtype=np.float32)
        slot_edge = np.full(S, -1, dtype=np.int64)
        for b in range(BLOCKS_PER_CORE):
            s0, e0 = starts[c, b], ends[c, b]
            k = e0 - s0
            o = block_off[b]
            rows = perm[s0:e0]
            slot_src[o : o + k] = src[rows]
            slot_rel[o : o + k] = (
                dst[rows] - (c * NODES_PER_CORE + b * P)
            ).astype(np.float32)
            slot_edge[o : o + k] = rows
        valid = slot_edge >= 0
        eT_np = np.zeros((P, S), dtype=np.float32)
        eT_np[:, valid] = (e[slot_edge[valid]] * 0.25).T

        hTblk_np = np.zeros((P, BLOCKS_PER_CORE * P), dtype=np.float32)
        ncols = min(BLOCKS_PER_CORE * P, NODES_PER_CORE)
        hTblk_np[:, :ncols] = h[c * NODES_PER_CORE : c * NODES_PER_CORE + ncols].T

        idx_np = np.empty((n_macros, P, 2 * MACRO), dtype=np.int32)
        idx_np[:, :, 0:MACRO] = slot_src.reshape(n_macros, MACRO, P).transpose(
            0, 2, 1
        )
        idx_np[:, :, MACRO : 2 * MACRO] = (
            slot_rel.view(np.int32).reshape(n_macros, MACRO, P).transpose(0, 2, 1)
        )
        in_maps.append(
            {
                "hT": hT_np,
                "hTblk": hTblk_np,
                "eT": eT_np,
                "idx": idx_np,
                "iota": iota_np,
                "ident": ident_np,
            }
        )
        eout_maps.append(slot_edge)
    return t_blocks, in_maps, eout_maps


_PROGRAM_CACHE = {}


def _run_spmd(nc, in_maps, core_ids):
    from concourse.bass_utils import run_bass_kernel_spmd

    return run_bass_kernel_spmd(nc, in_maps, core_ids)


def kernel(h, e, src, dst, Wq, bq, Wk, bk, Wv, bv, We, be):
    h = np.asarray(h, dtype=np.float32)
    e = np.asarray(e, dtype=np.float32)
    Wkv = np.ascontiguousarray(
        np.concatenate(
            [np.asarray(Wk, np.float32), np.asarray(Wv, np.float32)], axis=1
        )
    )
    Wq = np.ascontiguousarray(np.asarray(Wq, dtype=np.float32))
    We = np.ascontiguousarray(np.asarray(We, dtype=np.float32))
    bq = np.asarray(bq, dtype=np.float32)
    bk = np.asarray(bk, dtype=np.float32)
    bv = np.asarray(bv, dtype=np.float32)
    be = np.asarray(be, dtype=np.float32)
    use_bias = bool(
        np.abs(bq).max() or np.abs(bk).max() or np.abs(bv).max() or np.abs(be).max()
    )

    t_blocks, in_maps, eout_maps = _host_prep(h, e, src, dst)

    key = (tuple(t_blocks), use_bias)
    if key not in _PROGRAM_CACHE:
        _PROGRAM_CACHE[key] = _build_program(t_blocks, use_bias)
    nc = _PROGRAM_CACHE[key]

    bias_np = np.concatenate([bk, bv, bq, be * 0.25])[None, :].astype(np.float32)
    for m in in_maps:
        m["Wkv"] = Wkv
        m["Wq"] = Wq
        m["We"] = We
        if use_bias:
            m["bias"] = bias_np

    core_ids = list(range(NCORES))
    res = _run_spmd(nc, in_maps, core_ids)

    h_out = np.empty((N, HD), dtype=np.float32)
    e_out = np.empty((E, HD), dtype=np.float32)
    for c in core_ids:
        h_out[c * NODES_PER_CORE : (c + 1) * NODES_PER_CORE] = res.results[c]["hout"]
        sm = eout_maps[c]
        valid = sm >= 0
        e_out[sm[valid]] = res.results[c]["eout"][valid]
    return h_out.reshape(N, H, D), e_out.reshape(E, H, D)


# revision 36
# speedup vs baseline: 1.0055x; 1.0055x over previous
"""Trainium2 Bass kernel for GNN multi-head attention (gnn_message_passing).

Reference computation (per problem):
    Q = h @ Wq + bq; K = h @ Wk + bk; V = h @ Wv + bv       [N, H*D]
    pe = e @ We + be                                         [E, H*D]
    score = (K[src] * Q[dst]) / sqrt(D) * pe                 [E, H, D]  -> e_out
    att = exp(clip(sum_d score, -5, 5))                      [E, H, 1]
    wV = segment_sum(V[src] * att, dst, N); z = segment_sum(att, dst, N)
    h_out = wV / (z + 1e-6)                                  [N, H, D]

Strategy: edges are sorted by dst on the host and the 40000 destination nodes
are sharded across the 8 cores (5000 each), so each core's incoming edges are
a contiguous range of the sorted edge list and the segment-sum is core-local.
Per 128-node block the one-hot M[e, n] = (dst[e] == block_base + n) drives both
directions on the tensor engine: M^T expands the block's Q tile to per-edge
rows (so there is no Q gather and no Q table at all -- dst is block-local),
and M accumulates [V*att | att] into PSUM for the segment sum. K/V live in one
interleaved DRAM table built in phase 1, so a single indirect DMA per edge
tile gathers both (src is global). e arrives pre-transposed and pre-divided by
sqrt(D)=4 (an exact power-of-two fold) so pe is a plain matmul with the edge
tile as the stationary operand and e_out keeps full fp32 fidelity.
"""

import math

import numpy as np

# problem constants (hardcoded per contract - kernel.py must be self-contained)
N = 40000
E = 640000
IN = 128
H = 8
D = 16
HD = H * D  # 128
P = 128
NCORES = 8
NODES_PER_CORE = N // NCORES  # 5000
BLOCKS_PER_CORE = math.ceil(NODES_PER_CORE / P)  # 40 (39 full + 1 of 8 nodes)
MACRO = 16  # edge tiles per macro (gather/load batch)
NODE_PAD = 40064  # 313 * 128, node-table row padding
PH1_TILES = NODE_PAD // P  # 313


def _build_program(t_blocks, use_bias):
    """Build the SPMD Bass program. t_blocks: per-block tile counts,
    identical across cores (len BLOCKS_PER_CORE)."""
    import concourse.bacc as bacc
    import concourse.bass as bass
    import concourse.mybir as mybir
    import concourse.tile as tile

    f32 = mybir.dt.float32
    i32 = mybir.dt.int32
    Alu = mybir.AluOpType

    t_total = sum(t_blocks)
    assert t_total % MACRO == 0 and MACRO % 4 == 0
    n_macros = t_total // MACRO
    S = t_total * P  # edge slots per core

    nc = bacc.Bacc()

    # ---- parameters ----
    hT = nc.declare_dram_parameter("hT", [P, NODE_PAD], f32, isOutput=False)
    hTblk = nc.declare_dram_parameter(
        "hTblk", [P, BLOCKS_PER_CORE * P], f32, isOutput=False
    )
    eT = nc.declare_dram_parameter("eT", [P, S], f32, isOutput=False)
    idx = nc.declare_dram_parameter(
        "idx", [n_macros, P, 2 * MACRO], i32, isOutput=False
    )
    wkv = nc.declare_dram_parameter("Wkv", [IN, 2 * HD], f32, isOutput=False)
    wq = nc.declare_dram_parameter("Wq", [IN, HD], f32, isOutput=False)
    we = nc.declare_dram_parameter("We", [IN, HD], f32, isOutput=False)
    iota = nc.declare_dram_parameter("iota", [P, P], f32, isOutput=False)
    ident = nc.declare_dram_parameter("ident", [P, P], f32, isOutput=False)
    if use_bias:
        # [bk | bv | bq | be/4]
        bias = nc.declare_dram_parameter("bias", [1, 4 * HD], f32, isOutput=False)
    eout = nc.declare_dram_parameter("eout", [S, HD], f32, isOutput=True)
    hout = nc.declare_dram_parameter("hout", [NODES_PER_CORE, HD], f32, isOutput=True)

    # ---- internal DRAM K|V node table ----
    kvtab = nc.dram_tensor("kvtab", [NODE_PAD, 2 * HD], f32)

    with tile.TileContext(nc) as tc:
        with tc.tile_pool(name="const", bufs=1) as cpool:
            wkv_sb = cpool.tile([IN, 2 * HD], f32)
            wq_sb = cpool.tile([IN, HD], f32)
            we_sb = cpool.tile([IN, HD], f32)
            iota_sb = cpool.tile([P, P], f32)
            ident_sb = cpool.tile([P, P], f32)
            nc.sync.dma_start(out=wkv_sb[:], in_=wkv[:])
            nc.sync.dma_start(out=wq_sb[:], in_=wq[:])
            nc.sync.dma_start(out=we_sb[:], in_=we[:])
            nc.sync.dma_start(out=iota_sb[:], in_=iota[:])
            nc.sync.dma_start(out=ident_sb[:], in_=ident[:])
            if use_bias:
                bias_sb = cpool.tile([1, 4 * HD], f32)
                nc.sync.dma_start(out=bias_sb[:], in_=bias[:])
                ones_sb = cpool.tile([1, P], f32)
                nc.vector.memset(ones_sb[:], 1.0)

            # ---------------- phase 1: K|V node table ----------------
            with (
                tc.tile_pool(name="p1sb", bufs=3) as p1sb,
                tc.tile_pool(name="p1ps", bufs=4, space="PSUM") as p1ps,
                tc.tile_pool(name="p1out", bufs=3) as p1out,
            ):
                HMAC = 16  # node tiles per hT load / store group
                for i in range(PH1_TILES):
                    j = i % HMAC
                    if j == 0:
                        grp = min(HMAC, PH1_TILES - i)
                        w = grp * P
                        hl = p1sb.tile([P, HMAC * P], f32, tag="hl")
                        leng = nc.sync if (i // HMAC) % 2 == 1 else nc.scalar
                        leng.dma_start(out=hl[:, :w], in_=hT[:, i * P : i * P + w])
                        ob = p1out.tile([P, HMAC, 2 * HD], f32, tag="ob")
                    ps = p1ps.tile([P, 2 * HD], f32, space="PSUM")
                    nc.tensor.matmul(
                        out=ps[:],
                        lhsT=hl[:, j * P : (j + 1) * P],
                        rhs=wkv_sb[:],
                        start=True,
                        stop=use_bias is False,
                    )
                    if use_bias:
                        nc.tensor.matmul(
                            out=ps[:],
                            lhsT=ones_sb[:],
                            rhs=bias_sb[:, 0 : 2 * HD],
                            start=False,
                            stop=True,
                        )
                    nc.vector.tensor_copy(out=ob[:, j, :], in_=ps[:])
                    if j == grp - 1:
                        r0 = (i - j) * P
                        eng = nc.sync if (i // HMAC) % 2 == 0 else nc.scalar
                        eng.dma_start(
                            out=kvtab[r0 : r0 + grp * P, :].rearrange(
                                "(c p) f -> p c f", p=P
                            ),
                            in_=ob[:, :grp, :],
                        )

            # ---------------- phase 2: edges ----------------
            with (
                tc.tile_pool(name="idxp", bufs=3) as idxp,
                tc.tile_pool(name="etp", bufs=4) as etp,
                tc.tile_pool(name="kvp", bufs=4) as kvp,
                tc.tile_pool(name="eop", bufs=3) as eop,
                tc.tile_pool(name="qbp", bufs=2) as qbp,
                tc.tile_pool(name="pep", bufs=2, space="PSUM") as pep,
                tc.tile_pool(name="qep", bufs=3, space="PSUM") as qep,
                tc.tile_pool(name="mtqb", bufs=2, space="PSUM") as mtqb,
                tc.tile_pool(name="wvzp", bufs=1, space="PSUM") as wvzp,
                tc.tile_pool(name="work", bufs=5) as work,
            ):
                state = {"stripe": None, "hb": None, "qblk": None, "qb_b": None,
                         "wv": None}

                def _ensure_qblk(b):
                    if state["qb_b"] == b:
                        return
                    stripe = b // 4
                    if state["stripe"] != stripe:
                        hb = qbp.tile([P, 4 * P], f32, tag="hb")
                        w = min(4 * P, BLOCKS_PER_CORE * P - stripe * 4 * P)
                        nc.sync.dma_start(
                            out=hb[:, :w],
                            in_=hTblk[:, stripe * 4 * P : stripe * 4 * P + w],
                        )
                        state["stripe"] = stripe
                        state["hb"] = hb
                    qps = mtqb.tile([P, HD], f32, space="PSUM", tag="mtqb")
                    nc.tensor.matmul(
                        out=qps[:],
                        lhsT=state["hb"][:, (b % 4) * P : (b % 4 + 1) * P],
                        rhs=wq_sb[:],
                        start=True,
                        stop=use_bias is False,
                    )
                    if use_bias:
                        nc.tensor.matmul(
                            out=qps[:],
                            lhsT=ones_sb[:],
                            rhs=bias_sb[:, 2 * HD : 3 * HD],
                            start=False,
                            stop=True,
                        )
                    qsb = qbp.tile([P, HD], f32, tag="qblk")
                    nc.scalar.copy(out=qsb[:], in_=qps[:])
                    state["qblk"] = qsb
                    state["qb_b"] = b

                def _emit_block_end(b):
                    # h_out = wV / (z + 1e-6) for node block b
                    wv_ps = state["wv"]
                    nrows = min(P, NODES_PER_CORE - b * P)
                    zt = work.tile([P, H], f32, tag="zt")
                    nc.vector.tensor_scalar(
                        out=zt[:],
                        in0=wv_ps[:, HD : HD + H],
                        scalar1=1e-6,
                        scalar2=None,
                        op0=Alu.add,
                    )
                    zr = work.tile([P, H], f32, tag="zr")
                    nc.vector.reciprocal(out=zr[:], in_=zt[:])
                    ho = work.tile([P, HD], f32, tag="ho")
                    nc.vector.tensor_tensor(
                        out=ho[:].rearrange("p (h d) -> p h d", d=D),
                        in0=wv_ps[:, 0:HD].rearrange("p (h d) -> p h d", d=D),
                        in1=zr[:, :, None].to_broadcast([P, H, D]),
                        op=Alu.mult,
                    )
                    nc.sync.dma_start(
                        out=hout[b * P : b * P + nrows, :], in_=ho[:nrows, :]
                    )

                # flat tile list: (block, t_within_block, tb)
                tinfo = []
                for b in range(BLOCKS_PER_CORE):
                    for t in range(t_blocks[b]):
                        tinfo.append((b, t, t_blocks[b]))
                assert len(tinfo) % 4 == 0

                idx_sb = eT_sb = kv_sb = eo_sb = None
                for g4 in range(len(tinfo) // 4):
                    g0 = g4 * 4
                    m, j0 = divmod(g0, MACRO)
                    if j0 == 0:
                        idx_sb = idxp.tile([P, 2 * MACRO], i32, tag="idx")
                        nc.sync.dma_start(out=idx_sb[:], in_=idx[m])
                        eT_sb = etp.tile([P, MACRO * P], f32, tag="et")
                        nc.sync.dma_start(
                            out=eT_sb[:],
                            in_=eT[:, m * MACRO * P : (m + 1) * MACRO * P],
                        )
                        kv_sb = kvp.tile([P, MACRO, 2 * HD], f32, tag="kv")
                        for jj in range(MACRO):
                            nc.gpsimd.indirect_dma_start(
                                out=kv_sb[:, jj, :],
                                out_offset=None,
                                in_=kvtab[:],
                                in_offset=bass.IndirectOffsetOnAxis(
                                    ap=idx_sb[:, jj : jj + 1], axis=0
                                ),
                            )
                        eo_sb = eop.tile([P, MACRO * P], f32, tag="eo")
                        m4 = work.tile([P, MACRO * P], f32, tag="m4")
                        nc.vector.tensor_tensor(
                            out=m4[:].rearrange("p (c f) -> p c f", f=P),
                            in0=iota_sb[:, None, :].to_broadcast([P, MACRO, P]),
                            in1=idx_sb[:, MACRO : 2 * MACRO]
                            .bitcast(f32)[:, :, None]
                            .to_broadcast([P, MACRO, P]),
                            op=Alu.is_equal,
                        )

                    pe_ps = pep.tile([P, 4 * HD], f32, space="PSUM", tag="pe")
                    qe_ps = qep.tile([P, 4 * HD], f32, space="PSUM", tag="qe")
                    t4 = work.tile([P, 4 * P], f32, tag="t4")
                    va4 = work.tile([P, 4, HD + H], f32, tag="va4")
                    red4 = work.tile([P, 4 * H], f32, tag="red4")

                    for q4 in range(4):
                        j = j0 + q4
                        b, t, tb = tinfo[g0 + q4]
                        _ensure_qblk(b)
                        # pe' = (e/4) @ We
                        nc.tensor.matmul(
                            out=pe_ps[:, q4 * HD : (q4 + 1) * HD],
                            lhsT=eT_sb[:, j * P : (j + 1) * P],
                            rhs=we_sb[:],
                            start=True,
                            stop=use_bias is False,
                        )
                        if use_bias:
                            nc.tensor.matmul(
                                out=pe_ps[:, q4 * HD : (q4 + 1) * HD],
                                lhsT=ones_sb[:],
                                rhs=bias_sb[:, 3 * HD : 4 * HD],
                                start=False,
                                stop=True,
                            )
                        # M^T for this tile via PE transpose
                        mt_ps = mtqb.tile(
                            [P, P], f32, space="PSUM", tag="mtqb", name="mt_ps"
                        )
                        nc.tensor.transpose(
                            out=mt_ps[:],
                            in_=m4[:, j * P : (j + 1) * P],
                            identity=ident_sb[:],
                        )
                        mt_sb = work.tile([P, P], f32, tag="mt_sb")
                        nc.scalar.copy(out=mt_sb[:], in_=mt_ps[:])
                        # Q[dst] = M^T-expand of the block Q tile (exact row copy)
                        nc.tensor.matmul(
                            out=qe_ps[:, q4 * HD : (q4 + 1) * HD],
                            lhsT=mt_sb[:],
                            rhs=state["qblk"][:],
                            start=True,
                            stop=True,
                        )

                    # K*Q for 4 tiles
                    nc.vector.tensor_tensor(
                        out=t4[:].rearrange("p (c f) -> p c f", f=P),
                        in0=kv_sb[:, j0 : j0 + 4, 0:HD],
                        in1=qe_ps[:].rearrange("p (c f) -> p c f", f=P),
                        op=Alu.mult,
                    )
                    # score = (K*Q) * pe' for 4 tiles (this is the e_out data)
                    eo4 = eo_sb[:, j0 * P : (j0 + 4) * P]
                    nc.vector.tensor_tensor(
                        out=eo4, in0=t4[:], in1=pe_ps[:], op=Alu.mult
                    )
                    # att = exp(clip(sum_d score, -5, 5)) for 4 tiles
                    nc.vector.tensor_reduce(
                        out=red4[:],
                        in_=eo4.rearrange("p (c h d) -> p c h d", h=H, d=D),
                        axis=mybir.AxisListType.X,
                        op=Alu.add,
                    )
                    nc.vector.tensor_scalar(
                        out=red4[:],
                        in0=red4[:],
                        scalar1=5.0,
                        scalar2=-5.0,
                        op0=Alu.min,
                        op1=Alu.max,
                    )
                    nc.scalar.activation(
                        out=va4[:, :, HD : HD + H],
                        in_=red4[:].rearrange("p (c h) -> p c h", h=H),
                        func=mybir.ActivationFunctionType.Exp,
                    )
                    # V * att for 4 tiles (broadcast att over D)
                    nc.vector.tensor_tensor(
                        out=va4[:, :, 0:HD].rearrange("p c (h d) -> p c h d", d=D),
                        in0=kv_sb[:, j0 : j0 + 4, HD : 2 * HD].rearrange(
                            "p c (h d) -> p c h d", d=D
                        ),
                        in1=va4[:, :, HD : HD + H][:, :, :, None].to_broadcast(
                            [P, 4, H, D]
                        ),
                        op=Alu.mult,
                    )
                    if j0 == MACRO - 4:
                        # flush e_out macro: SBUF [P, MACRO, P] -> DRAM rows
                        nc.sync.dma_start(
                            out=eout[
                                m * MACRO * P : (m + 1) * MACRO * P, :
                            ].rearrange("(c p) f -> p c f", p=P),
                            in_=eo_sb[:].rearrange("p (c f) -> p c f", f=P),
                        )
                    # ---- per-tile segment-sum accumulate: [wV | z] ----
                    for q4 in range(4):
                        b, t, tb = tinfo[g0 + q4]
                        if t == 0:
                            state["wv"] = wvzp.tile(
                                [P, HD + H], f32, space="PSUM", tag="wv",
                                name="wv_ps",
                            )
                        last = t == tb - 1
                        nc.tensor.matmul(
                            out=state["wv"][:],
                            lhsT=m4[:, (j0 + q4) * P : (j0 + q4 + 1) * P],
                            rhs=va4[:, q4, :],
                            start=(t == 0),
                            stop=last,
                        )
                        if last:
                            _emit_block_end(b)

    nc.compile()
    return nc


def _host_prep(h, e, src, dst):
    """Sort/shard on host. Returns (t_blocks, in_maps, eout_maps)."""
    src = np.ascontiguousarray(src.astype(np.int32))
    dst = np.ascontiguousarray(dst.astype(np.int32))
    h = np.ascontiguousarray(h.astype(np.float32))
    e = np.ascontiguousarray(e.astype(np.float32))

    perm = np.argsort(dst, kind="stable")
    dst_s = dst[perm]

    starts = np.empty((NCORES, BLOCKS_PER_CORE), dtype=np.int64)
    ends = np.empty((NCORES, BLOCKS_PER_CORE), dtype=np.int64)
    for c in range(NCORES):
        lo = c * NODES_PER_CORE
        hi = (c + 1) * NODES_PER_CORE
        bl = np.minimum(lo + np.arange(BLOCKS_PER_CORE) * P, hi)
        bh = np.minimum(bl + P, hi)
        starts[c] = np.searchsorted(dst_s, bl)
        ends[c] = np.searchsorted(dst_s, bh)
    cnt = ends - starts  # [NCORES, BLOCKS_PER_CORE]
    t_blocks = np.maximum(1, ((cnt + P - 1) // P).max(axis=0))
    t_total = int(t_blocks.sum())
    pad = (-t_total) % MACRO
    t_blocks[-1] += pad
    t_total += pad
    t_blocks = [int(x) for x in t_blocks]
    S = t_total * P

    iota_np = np.tile(np.arange(P, dtype=np.float32), (P, 1))
    ident_np = np.eye(P, dtype=np.float32)
    hT_np = np.zeros((P, NODE_PAD), dtype=np.float32)
    hT_np[:, :N] = h.T

    in_maps = []
    eout_maps = []
    block_off = np.concatenate([[0], np.cumsum(np.array(t_blocks) * P)])
    n_macros = S // (MACRO * P)
    for c in range(NCORES):
        slot_src = np.zeros(S, dtype=np.int32)
        slot_rel = np.full(S, -1.0, dtype=np.float32)
        slot_edge = np.full(S, -1, dtype=np.int64)
        for b in range(BLOCKS_PER_CORE):
            s0, e0 = starts[c, b], ends[c, b]
            k = e0 - s0
            o = block_off[b]
            rows = perm[s0:e0]
            slot_src[o : o + k] = src[rows]
            slot_rel[o : o + k] = (
                dst[rows] - (c * NODES_PER_CORE + b * P)
            ).astype(np.float32)
            slot_edge[o : o + k] = rows
        valid = slot_edge >= 0
        eT_np = np.zeros((P, S), dtype=np.float32)
        eT_np[:, valid] = (e[slot_edge[valid]] * 0.25).T

        hTblk_np = np.zeros((P, BLOCKS_PER_CORE * P), dtype=np.float32)
        ncols = min(BLOCKS_PER_CORE * P, NODES_PER_CORE)
        hTblk_np[:, :ncols] = h[c * NODES_PER_CORE : c * NODES_PER_CORE + ncols].T

        idx_np = np.empty((n_macros, P, 2 * MACRO), dtype=np.int32)
        idx_np[:, :, 0:MACRO] = slot_src.reshape(n_macros, MACRO, P).transpose(
            0, 2, 1
        )
        idx_np[:, :, MACRO : 2 * MACRO] = (
            slot_rel.view(np.int32).reshape(n_macros, MACRO, P).transpose(0, 2, 1)
        )
        in_maps.append(
            {
                "hT": hT_np,
                "hTblk": hTblk_np,
                "eT": eT_np,
                "idx": idx_np,
                "iota": iota_np,
                "ident": ident_np,
            }
        )
        eout_maps.append(slot_edge)
    return t_blocks, in_maps, eout_maps


_PROGRAM_CACHE = {}


def _run_spmd(nc, in_maps, core_ids):
    from concourse.bass_utils import run_bass_kernel_spmd

    return run_bass_kernel_spmd(nc, in_maps, core_ids)


def kernel(h, e, src, dst, Wq, bq, Wk, bk, Wv, bv, We, be):
    h = np.asarray(h, dtype=np.float32)
    e = np.asarray(e, dtype=np.float32)
    Wkv = np.ascontiguousarray(
        np.concatenate(
            [np.asarray(Wk, np.float32), np.asarray(Wv, np.float32)], axis=1
        )
    )
    Wq = np.ascontiguousarray(np.asarray(Wq, dtype=np.float32))
    We = np.ascontiguousarray(np.asarray(We, dtype=np.float32))
    bq = np.asarray(bq, dtype=np.float32)
    bk = np.asarray(bk, dtype=np.float32)
    bv = np.asarray(bv, dtype=np.float32)
    be = np.asarray(be, dtype=np.float32)
    use_bias = bool(
        np.abs(bq).max() or np.abs(bk).max() or np.abs(bv).max() or np.abs(be).max()
    )

    t_blocks, in_maps, eout_maps = _host_prep(h, e, src, dst)

    key = (tuple(t_blocks), use_bias)
    if key not in _PROGRAM_CACHE:
        _PROGRAM_CACHE[key] = _build_program(t_blocks, use_bias)
    nc = _PROGRAM_CACHE[key]

    bias_np = np.concatenate([bk, bv, bq, be * 0.25])[None, :].astype(np.float32)
    for m in in_maps:
        m["Wkv"] = Wkv
        m["Wq"] = Wq
        m["We"] = We
        if use_bias:
            m["bias"] = bias_np

    core_ids = list(range(NCORES))
    res = _run_spmd(nc, in_maps, core_ids)

    h_out = np.empty((N, HD), dtype=np.float32)
    e_out = np.empty((E, HD), dtype=np.float32)
    for c in core_ids:
        h_out[c * NODES_PER_CORE : (c + 1) * NODES_PER_CORE] = res.results[c]["hout"]
        sm = eout_maps[c]
        valid = sm >= 0
        e_out[sm[valid]] = res.results[c]["eout"][valid]
    return h_out.reshape(N, H, D), e_out.reshape(E, H, D)
